# revision 18
# baseline (speedup 1.0000x reference)
"""GPT-2 attention block (QKV proj + causal attention w/ KV cache + out proj)
on 8 Trainium2 NeuronCores, tensor-parallel over heads (2 heads/core).

kernel(**inputs) takes FULL inputs, shards on host, runs one SPMD Bass/Tile
NEFF on cores 0-7 (per-batch AllToAll re-shards head-parallel attention
output to token-parallel for the output projection; batch-0's collective
overlaps batch-1's attention), and reassembles the full outputs on host.
"""
import sys

try:
    import concourse  # noqa: F401
except ImportError:
    sys.path.insert(0, "/opt/trn_rl_repo")

import numpy as np
import ml_dtypes

import concourse.bass as bass
import concourse.mybir as mybir
import concourse.tile as tile
from concourse import bacc
from concourse import bass_utils
from concourse.masks import make_upper_triangular, make_identity

BF16 = mybir.dt.bfloat16
F32 = mybir.dt.float32
NPBF16 = ml_dtypes.bfloat16

N_CORES = 8
B, S, D = 2, 2048, 1024
H, DH = 16, 64
HL = H // N_CORES          # heads per core = 2
P = 2048                   # past length
NS = P + S                 # 4096 keys
NKC = NS // 128            # 32 key chunks of 128
TOK = B * S                # 4096 global tokens
TPB = S // N_CORES         # 256 tokens per core per batch for out-proj
VSLOT = 130                # v_aug slot: [v_h0(64)|ones|v_h1(64)|ones]

_cache = {}


def build():
    nc = bacc.Bacc("TRN2", target_bir_lowering=False, debug=False,
                   num_devices=N_CORES)
    # ---- I/O ----
    xT = nc.dram_tensor("xT", [D, TOK], BF16, kind="ExternalInput").ap()
    w1s = nc.dram_tensor("w1s", [D, 6 * DH], BF16, kind="ExternalInput").ap()
    b1q = nc.dram_tensor("b1q", [2 * DH], F32, kind="ExternalInput").ap()
    b1k = nc.dram_tensor("b1k", [2 * DH], F32, kind="ExternalInput").ap()
    pastKT = nc.dram_tensor("pastKT", [B, 2 * DH, P], BF16, kind="ExternalInput").ap()
    pastV = nc.dram_tensor("pastV", [B, HL, P, DH], BF16, kind="ExternalInput").ap()
    w2 = nc.dram_tensor("w2", [D, D], BF16, kind="ExternalInput").ap()
    b2eff = nc.dram_tensor("b2eff", [D], F32, kind="ExternalInput").ap()
    sel2_in = nc.dram_tensor("sel2", [2, 128], BF16, kind="ExternalInput").ap()
    outT = nc.dram_tensor("outT", [D, B * TPB], F32, kind="ExternalOutput").ap()
    presK = nc.dram_tensor("presK", [B, HL, S, DH], F32, kind="ExternalOutput").ap()
    presV = nc.dram_tensor("presV", [B, HL, S, DH], F32, kind="ExternalOutput").ap()

    with tile.TileContext(nc) as tc:
        with tc.tile_pool(name="persist", bufs=1) as pp, \
             tc.tile_pool(name="xtp", bufs=3) as xtp, \
             tc.tile_pool(name="expp", bufs=3) as expp, \
             tc.tile_pool(name="stg", bufs=4) as stg, \
             tc.tile_pool(name="outp", bufs=2) as outp, \
             tc.tile_pool(name="psA", bufs=2, space="PSUM") as psA, \
             tc.tile_pool(name="psB", bufs=2, space="PSUM") as psB, \
             tc.tile_pool(name="dram", bufs=1, space="DRAM") as dramp:

            # ---- constants / persistent SBUF ----
            ident = pp.tile([128, 128], BF16)
            make_identity(nc, ident[:])
            umask = pp.tile([128, 128], BF16)  # umask[k, q] = 1 if k <= q
            make_upper_triangular(nc, umask[:], val=1.0)
            sel2 = pp.tile([2, 128], BF16)
            nc.sync.dma_start(sel2[:], sel2_in)

            b1q_sb = pp.tile([128, 1], F32)
            nc.sync.dma_start(b1q_sb[:], b1q.rearrange("(p o) -> p o", o=1))
            b1k_sb = pp.tile([128, 1], F32)
            nc.sync.dma_start(b1k_sb[:], b1k.rearrange("(p o) -> p o", o=1))
            b2e_sb = pp.tile([128, 8], F32)
            nc.sync.dma_start(b2e_sb[:], b2eff.rearrange("(o p) -> p o", p=128))
            w1_sb = pp.tile([128, 8, 6 * DH], BF16)
            nc.sync.dma_start(w1_sb[:], w1s.rearrange("(d p) c -> p d c", p=128))
            w2_sb = pp.tile([128, 8, D], BF16)
            nc.sync.dma_start(w2_sb[:], w2.rearrange("(f p) n -> p f n", p=128))

            kT_z = [[pp.tile([128, NS], BF16, name=f"kTz{b}{h}")
                     for h in range(HL)] for b in range(B)]
            qT_sb = [pp.tile([128, S], BF16, name=f"qT{b}") for b in range(B)]
            vaug = [pp.tile([128, NKC * VSLOT + 64], BF16, name=f"vaug{b}")
                    for b in range(B)]
            aT_all = pp.tile([128, TOK], BF16)
            sums_b = [pp.tile([4, 1024], F32, name=f"sums{b}") for b in range(B)]
            rec_b = [pp.tile([4, 1024], BF16, name=f"rec{b}") for b in range(B)]

            def vv2(b):
                return vaug[b][:, 0:NKC * VSLOT].rearrange(
                    "p (t c) -> p t c", c=VSLOT)

            for b in range(B):
                for h in range(HL):
                    r0 = h * 64  # head h's k lives at rows r0:r0+64 (q-aligned)
                    nc.sync.dma_start(kT_z[b][h][r0:r0 + 64, 0:P],
                                      pastKT[b, r0:r0 + 64, :])
                    nc.vector.memset(kT_z[b][h][64 - r0:128 - r0, :], 0.0)
                nc.vector.memset(vaug[b][:, NKC * VSLOT:], 0.0)
                vv = vv2(b)
                nc.vector.memset(vv[:, :, 64:65], 1.0)
                nc.vector.memset(vv[:, :, 129:130], 1.0)
                for h in range(HL):
                    off = 0 if h == 0 else 65
                    nc.sync.dma_start(
                        vv[:, 0:P // 128, off:off + 64],
                        pastV[b, h].rearrange("(t p) d -> p t d", p=128))

            # ---- QKV projection for one 512-token block ----
            def emit_qkv_block(b, tc4):
                t0 = b * S + tc4 * 512
                xt = xtp.tile([128, 8, 512], BF16, name="xt")
                nc.sync.dma_start(
                    xt[:], xT[:, t0:t0 + 512].rearrange("(d p) t -> p d t", p=128))
                for g in range(3):  # 0=q, 1=k, 2=v
                    ps = psB.tile([128, 1024], F32, tag="mm", name="qkvps")
                    for dc in range(8):
                        nc.tensor.matmul(
                            ps[:, 0:512],
                            lhsT=w1_sb[:, dc, g * 128:(g + 1) * 128],
                            rhs=xt[:, dc, :],
                            start=(dc == 0), stop=(dc == 7))
                    if g == 0:
                        nc.vector.tensor_scalar_add(
                            qT_sb[b][:, tc4 * 512:(tc4 + 1) * 512],
                            ps[:, 0:512], b1q_sb[:])
                        continue
                    if g == 1:
                        nc.vector.tensor_scalar_add(
                            kT_z[b][0][0:64, P + tc4 * 512:P + (tc4 + 1) * 512],
                            ps[0:64, 0:512], b1k_sb[0:64, :])
                        nc.vector.tensor_scalar_add(
                            kT_z[b][1][64:128, P + tc4 * 512:P + (tc4 + 1) * 512],
                            ps[64:128, 0:512], b1k_sb[64:128, :])
                    else:
                        vT_t = stg.tile([128, 512], BF16, tag="vT")
                        nc.vector.tensor_copy(vT_t[:], ps[:, 0:512])
                    for j in range(4):
                        t0l = tc4 * 512 + j * 128
                        if g == 1:
                            for h in range(HL):
                                r0 = h * 64
                                tp = psB.tile([128, 128], BF16, tag="mm",
                                              name="tpk")
                                nc.tensor.transpose(
                                    tp[:, 0:64],
                                    kT_z[b][h][r0:r0 + 64, P + t0l:P + t0l + 128],
                                    ident[r0:r0 + 64, r0:r0 + 64])
                                st = stg.tile([128, 64], F32, tag="st")
                                nc.vector.tensor_copy(st[:], tp[:, 0:64])
                                nc.sync.dma_start(
                                    presK[b, h, t0l:t0l + 128, :], st[:])
                        else:
                            tp = psB.tile([128, 128], BF16, tag="mm", name="tpv")
                            nc.tensor.transpose(
                                tp[:], vT_t[:, j * 128:(j + 1) * 128], ident[:])
                            st2 = stg.tile([128, 128], F32, tag="st2")
                            nc.vector.tensor_copy(st2[:], tp[:])
                            nc.sync.dma_start(
                                presV[b, :, t0l:t0l + 128, :].rearrange(
                                    "h t d -> t h d"),
                                st2[:].rearrange("t (h d) -> t h d", h=HL))
                            slot = P // 128 + tc4 * 4 + j
                            for h in range(HL):
                                off = 0 if h == 0 else 65
                                nc.vector.tensor_copy(
                                    vv2(b)[:, slot, off:off + 64],
                                    tp[:, h * 64:h * 64 + 64])

            def emit_scores(b, h, qbase, kc):
                kn0 = kc * 128 - P
                qlo = max(qbase, kn0)
                sc = psB.tile([128, 1024], F32, tag="mm", name=f"sc{b}{h}")
                for ss in (qbase, qbase + 512):
                    s = max(ss, qlo)
                    n = ss + 512 - s
                    if n <= 0:
                        continue
                    nc.tensor.matmul(
                        sc[:, s - qbase:s - qbase + n],
                        lhsT=kT_z[b][h][:, kc * 128:(kc + 1) * 128],
                        rhs=qT_sb[b][:, s:s + n],
                        start=True, stop=True)
                return sc

            # ---- attention for one (batch, head, 1024-query tile) unit;
            # kc loop software-pipelined so PE never waits on the exp ----
            def emit_attn_unit(b, h, qh):
                qbase = qh * 1024
                kc_end = min(NKC, (P + qbase + 1024) // 128)
                apsum = psA.tile([128, 1024], F32, tag="acc")
                sc = emit_scores(b, h, qbase, 0)
                for kc in range(kc_end):
                    kn0 = kc * 128 - P
                    qlo = max(qbase, kn0)
                    et = expp.tile([128, 1024], BF16, name="et")
                    nc.scalar.activation(
                        out=et[:, qlo - qbase:1024],
                        in_=sc[:, qlo - qbase:1024],
                        func=mybir.ActivationFunctionType.Exp,
                        scale=0.125)
                    if kc + 1 < kc_end:
                        sc = emit_scores(b, h, qbase, kc + 1)
                    if kn0 >= qbase:  # diagonal chunk: causal mask
                        nc.vector.tensor_mul(
                            et[:, kn0 - qbase:kn0 - qbase + 128],
                            et[:, kn0 - qbase:kn0 - qbase + 128],
                            umask[:])
                    voff = 0 if h == 0 else 65
                    for ss in (qbase, qbase + 512):
                        s = max(ss, qlo)
                        n = ss + 512 - s
                        if n <= 0:
                            continue
                        last_kc = min(kc_end - 1, (P + ss + 511) // 128)
                        nc.tensor.matmul(
                            apsum[:, s - qbase:s - qbase + n],
                            lhsT=vaug[b][:, kc * VSLOT + voff:
                                         kc * VSLOT + voff + 128],
                            rhs=et[:, s - qbase:s - qbase + n],
                            start=(kc == 0), stop=(kc == last_kc))
                # stash unnormalized aT (row 64 of apsum = softmax sums)
                nc.vector.tensor_copy(
                    aT_all[h * 64:(h + 1) * 64,
                           b * S + qbase:b * S + qbase + 1024],
                    apsum[0:64, :])
                su = stg.tile([1, 1024], F32, tag="sums", bufs=2)
                nc.vector.tensor_copy(su[:], apsum[64:65, :])
                nc.sync.dma_start(sums_b[b][qh * 2 + h:qh * 2 + h + 1, :], su[:])

            # ---- per-batch: normalize + AllToAll + output projection ----
            def emit_tail(b):
                with nc.allow_low_precision(reason="bf16 softmax recip"):
                    nc.vector.reciprocal(rec_b[b][:], sums_b[b][:])
                for qh in range(2):
                    rtmp = stg.tile([2, 1024], BF16, tag="rt", bufs=2)
                    nc.sync.dma_start(rtmp[:], rec_b[b][qh * 2:qh * 2 + 2, :])
                    rb = psB.tile([128, 1024], F32, tag="mm", name="rb")
                    for half in range(2):
                        nc.tensor.matmul(
                            rb[:, half * 512:(half + 1) * 512],
                            lhsT=sel2[:],
                            rhs=rtmp[:, half * 512:(half + 1) * 512],
                            start=True, stop=True)
                    rb_sb = stg.tile([128, 1024], F32, tag="rb", bufs=2)
                    nc.vector.tensor_copy(rb_sb[:], rb[:])
                    cols = slice(b * S + qh * 1024, b * S + qh * 1024 + 1024)
                    nc.vector.tensor_mul(
                        aT_all[:, cols], aT_all[:, cols], rb_sb[:])
                a2a_in = dramp.tile([N_CORES, 128, TPB], BF16, name=f"a2ai{b}")
                a2a_out = dramp.tile([N_CORES, 128, TPB], BF16, name=f"a2ao{b}")
                nc.sync.dma_start(
                    a2a_in.rearrange("r p t -> p r t"),
                    aT_all[:, b * S:(b + 1) * S].rearrange(
                        "p (r t) -> p r t", r=N_CORES))
                nc.gpsimd.collective_compute(
                    "AllToAll", mybir.AluOpType.bypass,
                    replica_groups=[list(range(N_CORES))],
                    ins=[a2a_in.opt()], outs=[a2a_out.opt()])
                ablk = pp.tile([128, N_CORES, TPB], BF16, name=f"ablk{b}")
                nc.sync.dma_start(ablk[:], a2a_out.rearrange("r p t -> p r t"))
                for oc in range(8):
                    op = psB.tile([128, 1024], F32, tag="mm", name="op")
                    for fc in range(8):
                        nc.tensor.matmul(
                            op[:, 0:TPB],
                            lhsT=w2_sb[:, fc, oc * 128:(oc + 1) * 128],
                            rhs=ablk[:, fc, :],
                            start=(fc == 0), stop=(fc == 7))
                    ot = outp.tile([128, TPB], F32)
                    nc.vector.tensor_scalar_add(
                        ot[:], op[:, 0:TPB], b2e_sb[:, oc:oc + 1])
                    nc.sync.dma_start(
                        outT[oc * 128:(oc + 1) * 128, b * TPB:(b + 1) * TPB],
                        ot[:])

            # ---- emission schedule: qkv(b0); then qkv(b1) interleaved with
            # attention(b0); tail(b0) overlaps attention(b1); tail(b1) ----
            units = [(h, qh) for h in range(HL) for qh in range(2)]
            for tc4 in range(4):
                emit_qkv_block(0, tc4)
            for i in range(4):
                emit_qkv_block(1, i)
                emit_attn_unit(0, *units[i])
            emit_tail(0)
            for h, qh in units:
                emit_attn_unit(1, h, qh)
            emit_tail(1)

    nc.compile()
    return nc


def _get_nc():
    if "nc" not in _cache:
        _cache["nc"] = build()
    return _cache["nc"]


def kernel(x, past, w1, b1, w2, b2):
    x = np.asarray(x, dtype=np.float32)
    past = np.asarray(past, dtype=np.float32)
    w1 = np.asarray(w1, dtype=np.float32)
    b1 = np.asarray(b1, dtype=np.float32)
    w2 = np.asarray(w2, dtype=np.float32)
    b2 = np.asarray(b2, dtype=np.float32)

    xT = np.ascontiguousarray(x.reshape(TOK, D).T).astype(NPBF16)
    w2b = np.ascontiguousarray(w2).astype(NPBF16)
    b1v_full = b1[2 * D:3 * D]
    b2eff = (b2 + b1v_full.astype(np.float64) @ w2.astype(np.float64)).astype(np.float32)
    sel2 = np.zeros((2, 128), dtype=NPBF16)
    sel2[0, 0:64] = 1
    sel2[1, 64:128] = 1

    in_maps = []
    for c in range(N_CORES):
        hs = [2 * c, 2 * c + 1]
        qcols = np.concatenate([w1[:, h * DH:(h + 1) * DH] for h in hs], axis=1)
        kcols = np.concatenate([w1[:, D + h * DH:D + (h + 1) * DH] for h in hs], axis=1)
        vcols = np.concatenate([w1[:, 2 * D + h * DH:2 * D + (h + 1) * DH] for h in hs], axis=1)
        w1sc = np.concatenate([qcols, kcols, vcols], axis=1).astype(NPBF16)
        b1qc = np.concatenate([b1[h * DH:(h + 1) * DH] for h in hs])
        b1kc = np.concatenate([b1[D + h * DH:D + (h + 1) * DH] for h in hs])
        pkT = np.stack([np.concatenate(
            [past[b, 0, h].T for h in hs], axis=0) for b in range(B)]).astype(NPBF16)
        pv = np.stack([np.stack([past[b, 1, h] for h in hs])
                       for b in range(B)]).astype(NPBF16)
        in_maps.append({
            "xT": xT, "w1s": np.ascontiguousarray(w1sc), "sel2": sel2,
            "b1q": np.ascontiguousarray(b1qc).astype(np.float32),
            "b1k": np.ascontiguousarray(b1kc).astype(np.float32),
            "pastKT": np.ascontiguousarray(pkT),
            "pastV": np.ascontiguousarray(pv),
            "w2": w2b, "b2eff": b2eff,
        })

    _cache["last_in_maps"] = in_maps
    nc = _get_nc()
    res = bass_utils.run_bass_kernel_spmd(
        nc, in_maps, core_ids=list(range(N_CORES)), trace=False)
    _cache["last_results"] = res

    out_flat = np.empty((TOK, D), dtype=np.float32)
    present = np.empty((B, 2, H, S, DH), dtype=np.float32)
    for c in range(N_CORES):
        r = res.results[c]
        for b in range(B):
            out_flat[b * S + c * TPB:b * S + (c + 1) * TPB, :] = \
                r["outT"][:, b * TPB:(b + 1) * TPB].T
        for j, h in enumerate((2 * c, 2 * c + 1)):
            present[:, 0, h] = r["presK"][:, j]
            present[:, 1, h] = r["presV"][:, j] + b1[2 * D + h * DH:2 * D + (h + 1) * DH]
    out = out_flat.reshape(B, S, D)
    return out, present


# revision 21
# speedup vs baseline: 1.1032x; 1.1032x over previous
"""GPT-2 attention block (QKV proj + causal attention w/ KV cache + out proj)
on 8 Trainium2 NeuronCores, tensor-parallel over heads (2 heads/core).

kernel(**inputs) takes FULL inputs, shards on host, runs one SPMD Bass/Tile
NEFF on cores 0-7 (per-batch AllToAll re-shards head-parallel attention
output to token-parallel for the output projection; batch-0's collective
overlaps batch-1's attention), and reassembles the full outputs on host.
"""
import sys

try:
    import concourse  # noqa: F401
except ImportError:
    sys.path.insert(0, "/opt/trn_rl_repo")

import numpy as np
import ml_dtypes

import concourse.bass as bass
import concourse.mybir as mybir
import concourse.tile as tile
from concourse import bacc
from concourse import bass_utils
from concourse.masks import make_upper_triangular, make_identity

BF16 = mybir.dt.bfloat16
F32 = mybir.dt.float32
NPBF16 = ml_dtypes.bfloat16

N_CORES = 8
B, S, D = 2, 2048, 1024
H, DH = 16, 64
HL = H // N_CORES          # heads per core = 2
P = 2048                   # past length
NS = P + S                 # 4096 keys
NKC = NS // 128            # 32 key chunks of 128
TOK = B * S                # 4096 global tokens
TPB = S // N_CORES         # 256 tokens per core per batch for out-proj
VSLOT = 130                # v_aug slot: [v_h0(64)|ones|v_h1(64)|ones]

_cache = {}


def build():
    nc = bacc.Bacc("TRN2", target_bir_lowering=False, debug=False,
                   num_devices=N_CORES)
    # ---- I/O ----
    xT = nc.dram_tensor("xT", [D, TOK], BF16, kind="ExternalInput").ap()
    w1s = nc.dram_tensor("w1s", [D, 6 * DH], BF16, kind="ExternalInput").ap()
    b1q = nc.dram_tensor("b1q", [2 * DH], F32, kind="ExternalInput").ap()
    b1k = nc.dram_tensor("b1k", [2 * DH], F32, kind="ExternalInput").ap()
    pastKT = nc.dram_tensor("pastKT", [B, 2 * DH, P], BF16, kind="ExternalInput").ap()
    pastV = nc.dram_tensor("pastV", [B, HL, P, DH], BF16, kind="ExternalInput").ap()
    w2 = nc.dram_tensor("w2", [D, D], BF16, kind="ExternalInput").ap()
    b2eff = nc.dram_tensor("b2eff", [D], F32, kind="ExternalInput").ap()
    outT = nc.dram_tensor("outT", [D, B * TPB], F32, kind="ExternalOutput").ap()
    presK = nc.dram_tensor("presK", [B, HL, S, DH], F32, kind="ExternalOutput").ap()
    presV = nc.dram_tensor("presV", [B, HL, S, DH], F32, kind="ExternalOutput").ap()

    with tile.TileContext(nc) as tc:
        with tc.tile_pool(name="persist", bufs=1) as pp, \
             tc.tile_pool(name="xtp", bufs=3) as xtp, \
             tc.tile_pool(name="expp", bufs=4) as expp, \
             tc.tile_pool(name="stg", bufs=4) as stg, \
             tc.tile_pool(name="outp", bufs=2) as outp, \
             tc.tile_pool(name="psA", bufs=2, space="PSUM") as psA, \
             tc.tile_pool(name="psB", bufs=2, space="PSUM") as psB, \
             tc.tile_pool(name="dram", bufs=1, space="DRAM") as dramp:

            # ---- constants / persistent SBUF ----
            ident = pp.tile([128, 128], BF16)
            make_identity(nc, ident[:])
            umask = pp.tile([128, 128], BF16)  # umask[k, q] = 1 if k <= q
            make_upper_triangular(nc, umask[:], val=1.0)

            b1q_sb = pp.tile([128, 1], F32)
            nc.sync.dma_start(b1q_sb[:], b1q.rearrange("(p o) -> p o", o=1))
            b1k_sb = pp.tile([128, 1], F32)
            nc.sync.dma_start(b1k_sb[:], b1k.rearrange("(p o) -> p o", o=1))
            b2e_sb = pp.tile([128, 8], F32)
            nc.sync.dma_start(b2e_sb[:], b2eff.rearrange("(o p) -> p o", p=128))
            w1_sb = pp.tile([128, 8, 6 * DH], BF16)
            nc.sync.dma_start(w1_sb[:], w1s.rearrange("(d p) c -> p d c", p=128))
            w2_sb = pp.tile([128, 8, D], BF16)
            nc.sync.dma_start(w2_sb[:], w2.rearrange("(f p) n -> p f n", p=128))

            kT_z = [[pp.tile([128, NS], BF16, name=f"kTz{b}{h}")
                     for h in range(HL)] for b in range(B)]
            qT_sb = [pp.tile([128, S], BF16, name=f"qT{b}") for b in range(B)]
            vaug = [pp.tile([128, NKC * VSLOT + 64], BF16, name=f"vaug{b}")
                    for b in range(B)]
            aT_all = pp.tile([128, TOK], BF16)
            sums_b = [pp.tile([4, 1024], F32, name=f"sums{b}") for b in range(B)]
            rec_b = [pp.tile([4, 1024], BF16, name=f"rec{b}") for b in range(B)]

            def vv2(b):
                return vaug[b][:, 0:NKC * VSLOT].rearrange(
                    "p (t c) -> p t c", c=VSLOT)

            for b in range(B):
                for h in range(HL):
                    r0 = h * 64  # head h's k lives at rows r0:r0+64 (q-aligned)
                    nc.sync.dma_start(kT_z[b][h][r0:r0 + 64, 0:P],
                                      pastKT[b, r0:r0 + 64, :])
                    nc.vector.memset(kT_z[b][h][64 - r0:128 - r0, :], 0.0)
                nc.vector.memset(vaug[b][:, NKC * VSLOT:], 0.0)
                vv = vv2(b)
                nc.vector.memset(vv[:, :, 64:65], 1.0)
                nc.vector.memset(vv[:, :, 129:130], 1.0)
                for h in range(HL):
                    off = 0 if h == 0 else 65
                    nc.sync.dma_start(
                        vv[:, 0:P // 128, off:off + 64],
                        pastV[b, h].rearrange("(t p) d -> p t d", p=128))

            # ---- QKV projection for one 512-token block ----
            def emit_qkv_block(b, tc4):
                t0 = b * S + tc4 * 512
                xt = xtp.tile([128, 8, 512], BF16, name="xt")
                nc.sync.dma_start(
                    xt[:], xT[:, t0:t0 + 512].rearrange("(d p) t -> p d t", p=128))
                for g in range(3):  # 0=q, 1=k, 2=v
                    ps = psB.tile([128, 1024], F32, tag="mm", name="qkvps")
                    for dc in range(8):
                        nc.tensor.matmul(
                            ps[:, 0:512],
                            lhsT=w1_sb[:, dc, g * 128:(g + 1) * 128],
                            rhs=xt[:, dc, :],
                            start=(dc == 0), stop=(dc == 7))
                    if g == 0:
                        nc.vector.tensor_scalar_add(
                            qT_sb[b][:, tc4 * 512:(tc4 + 1) * 512],
                            ps[:, 0:512], b1q_sb[:])
                        continue
                    if g == 1:
                        nc.vector.tensor_scalar_add(
                            kT_z[b][0][0:64, P + tc4 * 512:P + (tc4 + 1) * 512],
                            ps[0:64, 0:512], b1k_sb[0:64, :])
                        nc.vector.tensor_scalar_add(
                            kT_z[b][1][64:128, P + tc4 * 512:P + (tc4 + 1) * 512],
                            ps[64:128, 0:512], b1k_sb[64:128, :])
                    else:
                        vT_t = stg.tile([128, 512], BF16, tag="vT")
                        nc.vector.tensor_copy(vT_t[:], ps[:, 0:512])
                    for j in range(4):
                        t0l = tc4 * 512 + j * 128
                        if g == 1:
                            for h in range(HL):
                                r0 = h * 64
                                tp = psB.tile([128, 128], BF16, tag="mm",
                                              name="tpk")
                                nc.tensor.transpose(
                                    tp[:, 0:64],
                                    kT_z[b][h][r0:r0 + 64, P + t0l:P + t0l + 128],
                                    ident[r0:r0 + 64, r0:r0 + 64])
                                st = stg.tile([128, 64], F32, tag="st")
                                nc.vector.tensor_copy(st[:], tp[:, 0:64])
                                nc.sync.dma_start(
                                    presK[b, h, t0l:t0l + 128, :], st[:])
                        else:
                            tp = psB.tile([128, 128], BF16, tag="mm", name="tpv")
                            nc.tensor.transpose(
                                tp[:], vT_t[:, j * 128:(j + 1) * 128], ident[:])
                            st2 = stg.tile([128, 128], F32, tag="st2")
                            nc.vector.tensor_copy(st2[:], tp[:])
                            nc.sync.dma_start(
                                presV[b, :, t0l:t0l + 128, :].rearrange(
                                    "h t d -> t h d"),
                                st2[:].rearrange("t (h d) -> t h d", h=HL))
                            slot = P // 128 + tc4 * 4 + j
                            for h in range(HL):
                                off = 0 if h == 0 else 65
                                nc.vector.tensor_copy(
                                    vv2(b)[:, slot, off:off + 64],
                                    tp[:, h * 64:h * 64 + 64])

            def emit_scores(b, h, qbase, kc):
                kn0 = kc * 128 - P
                qlo = max(qbase, kn0)
                sc = psB.tile([128, 1024], F32, tag="mm", name=f"sc{b}{h}")
                for ss in (qbase, qbase + 512):
                    s = max(ss, qlo)
                    n = ss + 512 - s
                    if n <= 0:
                        continue
                    nc.tensor.matmul(
                        sc[:, s - qbase:s - qbase + n],
                        lhsT=kT_z[b][h][:, kc * 128:(kc + 1) * 128],
                        rhs=qT_sb[b][:, s:s + n],
                        start=True, stop=True)
                return sc

            # ---- attention for one (batch, head, 1024-query tile) unit;
            # kc loop software-pipelined so PE never waits on the exp ----
            def emit_attn_unit(b, h, qh):
                qbase = qh * 1024
                kc_end = min(NKC, (P + qbase + 1024) // 128)
                apsum = psA.tile([128, 1024], F32, tag="acc")
                sc = emit_scores(b, h, qbase, 0)
                for kc in range(kc_end):
                    kn0 = kc * 128 - P
                    qlo = max(qbase, kn0)
                    et = expp.tile([128, 1024], BF16, name="et")
                    nc.scalar.activation(
                        out=et[:, qlo - qbase:1024],
                        in_=sc[:, qlo - qbase:1024],
                        func=mybir.ActivationFunctionType.Exp,
                        scale=0.125)
                    if kc + 1 < kc_end:
                        sc = emit_scores(b, h, qbase, kc + 1)
                    if kn0 >= qbase:  # diagonal chunk: causal mask
                        nc.vector.tensor_mul(
                            et[:, kn0 - qbase:kn0 - qbase + 128],
                            et[:, kn0 - qbase:kn0 - qbase + 128],
                            umask[:])
                    voff = 0 if h == 0 else 65
                    for ss in (qbase, qbase + 512):
                        s = max(ss, qlo)
                        n = ss + 512 - s
                        if n <= 0:
                            continue
                        last_kc = min(kc_end - 1, (P + ss + 511) // 128)
                        nc.tensor.matmul(
                            apsum[:, s - qbase:s - qbase + n],
                            lhsT=vaug[b][:, kc * VSLOT + voff:
                                         kc * VSLOT + voff + 128],
                            rhs=et[:, s - qbase:s - qbase + n],
                            start=(kc == 0), stop=(kc == last_kc))
                # stash unnormalized aT (row 64 of apsum = softmax sums)
                nc.vector.tensor_copy(
                    aT_all[h * 64:(h + 1) * 64,
                           b * S + qbase:b * S + qbase + 1024],
                    apsum[0:64, :])
                su = stg.tile([1, 1024], F32, tag="sums", bufs=2)
                nc.vector.tensor_copy(su[:], apsum[64:65, :])
                nc.sync.dma_start(sums_b[b][qh * 2 + h:qh * 2 + h + 1, :], su[:])

            # ---- per-batch: normalize (DVE + DMA broadcast, no PE/PSUM),
            # AllToAll, and output projection ----
            rec_dram = [dramp.tile([4, 1024], BF16, name=f"recd{b}")
                        for b in range(B)]
            a2a_in = [dramp.tile([N_CORES, 128, TPB], BF16, name=f"a2ai{b}")
                      for b in range(B)]
            a2a_out = [dramp.tile([N_CORES, 128, TPB], BF16, name=f"a2ao{b}")
                       for b in range(B)]

            def emit_norm(b):
                with nc.allow_low_precision(reason="bf16 softmax recip"):
                    nc.vector.reciprocal(rec_b[b][:], sums_b[b][:])
                nc.sync.dma_start(rec_dram[b].opt(), rec_b[b][:])
                for qh in range(2):
                    rb_sb = stg.tile([128, 1024], BF16, tag="rb", bufs=2)
                    for h in range(HL):
                        u = qh * 2 + h
                        nc.sync.dma_start(
                            rb_sb[h * 64:(h + 1) * 64, :].rearrange(
                                "r (o t) -> r o t", o=1),
                            rec_dram[b][u:u + 1, :].partition_broadcast(64))
                    cols = slice(b * S + qh * 1024, b * S + qh * 1024 + 1024)
                    nc.vector.tensor_mul(
                        aT_all[:, cols], aT_all[:, cols], rb_sb[:])

            def emit_a2a(b):
                nc.sync.dma_start(
                    a2a_in[b].rearrange("r p t -> p r t"),
                    aT_all[:, b * S:(b + 1) * S].rearrange(
                        "p (r t) -> p r t", r=N_CORES))
                nc.gpsimd.collective_compute(
                    "AllToAll", mybir.AluOpType.bypass,
                    replica_groups=[list(range(N_CORES))],
                    ins=[a2a_in[b].opt()], outs=[a2a_out[b].opt()])

            def emit_outproj(b):
                ablk = pp.tile([128, N_CORES, TPB], BF16, name=f"ablk{b}")
                nc.sync.dma_start(ablk[:], a2a_out[b].rearrange("r p t -> p r t"))
                for oc in range(8):
                    op = psB.tile([128, 1024], F32, tag="mm", name="op")
                    for fc in range(8):
                        nc.tensor.matmul(
                            op[:, 0:TPB],
                            lhsT=w2_sb[:, fc, oc * 128:(oc + 1) * 128],
                            rhs=ablk[:, fc, :],
                            start=(fc == 0), stop=(fc == 7))
                    ot = outp.tile([128, TPB], F32)
                    nc.vector.tensor_scalar_add(
                        ot[:], op[:, 0:TPB], b2e_sb[:, oc:oc + 1])
                    nc.sync.dma_start(
                        outT[oc * 128:(oc + 1) * 128, b * TPB:(b + 1) * TPB],
                        ot[:])

            # ---- emission schedule: qkv(b0); qkv(b1) interleaved with
            # attention(b0); norm+a2a(b0); attention(b1) (hides b0's
            # collective); outproj(b0) mid-attention; norm+a2a(b1); outproj ----
            units = [(h, qh) for h in range(HL) for qh in range(2)]
            for tc4 in range(4):
                emit_qkv_block(0, tc4)
            for i in range(4):
                emit_qkv_block(1, i)
                emit_attn_unit(0, *units[i])
            emit_norm(0)
            emit_a2a(0)
            emit_attn_unit(1, *units[0])
            emit_attn_unit(1, *units[1])
            emit_outproj(0)
            emit_attn_unit(1, *units[2])
            emit_attn_unit(1, *units[3])
            emit_norm(1)
            emit_a2a(1)
            emit_outproj(1)

    nc.compile()
    return nc


def _get_nc():
    if "nc" not in _cache:
        _cache["nc"] = build()
    return _cache["nc"]


def kernel(x, past, w1, b1, w2, b2):
    x = np.asarray(x, dtype=np.float32)
    past = np.asarray(past, dtype=np.float32)
    w1 = np.asarray(w1, dtype=np.float32)
    b1 = np.asarray(b1, dtype=np.float32)
    w2 = np.asarray(w2, dtype=np.float32)
    b2 = np.asarray(b2, dtype=np.float32)

    xT = np.ascontiguousarray(x.reshape(TOK, D).T).astype(NPBF16)
    w2b = np.ascontiguousarray(w2).astype(NPBF16)
    b1v_full = b1[2 * D:3 * D]
    b2eff = (b2 + b1v_full.astype(np.float64) @ w2.astype(np.float64)).astype(np.float32)

    in_maps = []
    for c in range(N_CORES):
        hs = [2 * c, 2 * c + 1]
        qcols = np.concatenate([w1[:, h * DH:(h + 1) * DH] for h in hs], axis=1)
        kcols = np.concatenate([w1[:, D + h * DH:D + (h + 1) * DH] for h in hs], axis=1)
        vcols = np.concatenate([w1[:, 2 * D + h * DH:2 * D + (h + 1) * DH] for h in hs], axis=1)
        w1sc = np.concatenate([qcols, kcols, vcols], axis=1).astype(NPBF16)
        b1qc = np.concatenate([b1[h * DH:(h + 1) * DH] for h in hs])
        b1kc = np.concatenate([b1[D + h * DH:D + (h + 1) * DH] for h in hs])
        pkT = np.stack([np.concatenate(
            [past[b, 0, h].T for h in hs], axis=0) for b in range(B)]).astype(NPBF16)
        pv = np.stack([np.stack([past[b, 1, h] for h in hs])
                       for b in range(B)]).astype(NPBF16)
        in_maps.append({
            "xT": xT, "w1s": np.ascontiguousarray(w1sc),
            "b1q": np.ascontiguousarray(b1qc).astype(np.float32),
            "b1k": np.ascontiguousarray(b1kc).astype(np.float32),
            "pastKT": np.ascontiguousarray(pkT),
            "pastV": np.ascontiguousarray(pv),
            "w2": w2b, "b2eff": b2eff,
        })

    _cache["last_in_maps"] = in_maps
    nc = _get_nc()
    res = bass_utils.run_bass_kernel_spmd(
        nc, in_maps, core_ids=list(range(N_CORES)), trace=False)
    _cache["last_results"] = res

    out_flat = np.empty((TOK, D), dtype=np.float32)
    present = np.empty((B, 2, H, S, DH), dtype=np.float32)
    for c in range(N_CORES):
        r = res.results[c]
        for b in range(B):
            out_flat[b * S + c * TPB:b * S + (c + 1) * TPB, :] = \
                r["outT"][:, b * TPB:(b + 1) * TPB].T
        for j, h in enumerate((2 * c, 2 * c + 1)):
            present[:, 0, h] = r["presK"][:, j]
            present[:, 1, h] = r["presV"][:, j] + b1[2 * D + h * DH:2 * D + (h + 1) * DH]
    out = out_flat.reshape(B, S, D)
    return out, present


# revision 22
# speedup vs baseline: 1.1287x; 1.0231x over previous
"""GPT-2 attention block (QKV proj + causal attention w/ KV cache + out proj)
on 8 Trainium2 NeuronCores, tensor-parallel over heads (2 heads/core).

kernel(**inputs) takes FULL inputs, shards on host, runs one SPMD Bass/Tile
NEFF on cores 0-7 (per-batch AllToAll re-shards head-parallel attention
output to token-parallel for the output projection; batch-0's collective
overlaps batch-1's attention), and reassembles the full outputs on host.
"""
import sys

try:
    import concourse  # noqa: F401
except ImportError:
    sys.path.insert(0, "/opt/trn_rl_repo")

import numpy as np
import ml_dtypes

import concourse.bass as bass
import concourse.mybir as mybir
import concourse.tile as tile
from concourse import bacc
from concourse import bass_utils
from concourse.masks import make_upper_triangular, make_identity

BF16 = mybir.dt.bfloat16
F32 = mybir.dt.float32
NPBF16 = ml_dtypes.bfloat16

N_CORES = 8
B, S, D = 2, 2048, 1024
H, DH = 16, 64
HL = H // N_CORES          # heads per core = 2
P = 2048                   # past length
NS = P + S                 # 4096 keys
NKC = NS // 128            # 32 key chunks of 128
TOK = B * S                # 4096 global tokens
TPB = S // N_CORES         # 256 tokens per core per batch for out-proj
VSLOT = 130                # v_aug slot: [v_h0(64)|ones|v_h1(64)|ones]

_cache = {}


def build():
    nc = bacc.Bacc("TRN2", target_bir_lowering=False, debug=False,
                   num_devices=N_CORES)
    # ---- I/O ----
    xT = nc.dram_tensor("xT", [D, TOK], BF16, kind="ExternalInput").ap()
    w1s = nc.dram_tensor("w1s", [D, 6 * DH], BF16, kind="ExternalInput").ap()
    b1q = nc.dram_tensor("b1q", [2 * DH], F32, kind="ExternalInput").ap()
    b1k = nc.dram_tensor("b1k", [2 * DH], F32, kind="ExternalInput").ap()
    pastKT = nc.dram_tensor("pastKT", [B, 2 * DH, P], BF16, kind="ExternalInput").ap()
    pastV = nc.dram_tensor("pastV", [B, HL, P, DH], BF16, kind="ExternalInput").ap()
    w2 = nc.dram_tensor("w2", [D, D], BF16, kind="ExternalInput").ap()
    b2eff = nc.dram_tensor("b2eff", [D], F32, kind="ExternalInput").ap()
    outT = nc.dram_tensor("outT", [D, B * TPB], F32, kind="ExternalOutput").ap()
    presK = nc.dram_tensor("presK", [B, HL, S, DH], F32, kind="ExternalOutput").ap()
    presV = nc.dram_tensor("presV", [B, HL, S, DH], F32, kind="ExternalOutput").ap()

    with tile.TileContext(nc) as tc:
        with tc.tile_pool(name="persist", bufs=1) as pp, \
             tc.tile_pool(name="xtp", bufs=3) as xtp, \
             tc.tile_pool(name="expp", bufs=4) as expp, \
             tc.tile_pool(name="stg", bufs=4) as stg, \
             tc.tile_pool(name="outp", bufs=2) as outp, \
             tc.tile_pool(name="psA", bufs=2, space="PSUM") as psA, \
             tc.tile_pool(name="psB", bufs=2, space="PSUM") as psB, \
             tc.tile_pool(name="dram", bufs=1, space="DRAM") as dramp:

            # ---- constants / persistent SBUF ----
            ident = pp.tile([128, 128], BF16)
            make_identity(nc, ident[:])
            umask = pp.tile([128, 128], BF16)  # umask[k, q] = 1 if k <= q
            make_upper_triangular(nc, umask[:], val=1.0)

            b1q_sb = pp.tile([128, 1], F32)
            nc.sync.dma_start(b1q_sb[:], b1q.rearrange("(p o) -> p o", o=1))
            b1k_sb = pp.tile([128, 1], F32)
            nc.sync.dma_start(b1k_sb[:], b1k.rearrange("(p o) -> p o", o=1))
            b2e_sb = pp.tile([128, 8], F32)
            nc.sync.dma_start(b2e_sb[:], b2eff.rearrange("(o p) -> p o", p=128))
            w1_sb = pp.tile([128, 8, 6 * DH], BF16)
            nc.sync.dma_start(w1_sb[:], w1s.rearrange("(d p) c -> p d c", p=128))
            w2_sb = pp.tile([128, 8, D], BF16)
            nc.sync.dma_start(w2_sb[:], w2.rearrange("(f p) n -> p f n", p=128))

            kT_z = [[pp.tile([128, NS], BF16, name=f"kTz{b}{h}")
                     for h in range(HL)] for b in range(B)]
            qT_sb = [pp.tile([128, S], BF16, name=f"qT{b}") for b in range(B)]
            vaug = [pp.tile([128, NKC * VSLOT + 64], BF16, name=f"vaug{b}")
                    for b in range(B)]
            aT_all = pp.tile([128, TOK], BF16)
            sums_bq = [[pp.tile([2, 1024], F32, name=f"sums{b}{qh}")
                        for qh in range(2)] for b in range(B)]
            rec_bq = [[pp.tile([2, 1024], BF16, name=f"rec{b}{qh}")
                       for qh in range(2)] for b in range(B)]

            def vv2(b):
                return vaug[b][:, 0:NKC * VSLOT].rearrange(
                    "p (t c) -> p t c", c=VSLOT)

            for b in range(B):
                for h in range(HL):
                    r0 = h * 64  # head h's k lives at rows r0:r0+64 (q-aligned)
                    nc.sync.dma_start(kT_z[b][h][r0:r0 + 64, 0:P],
                                      pastKT[b, r0:r0 + 64, :])
                    nc.vector.memset(kT_z[b][h][64 - r0:128 - r0, :], 0.0)
                nc.vector.memset(vaug[b][:, NKC * VSLOT:], 0.0)
                vv = vv2(b)
                nc.vector.memset(vv[:, :, 64:65], 1.0)
                nc.vector.memset(vv[:, :, 129:130], 1.0)
                for h in range(HL):
                    off = 0 if h == 0 else 65
                    nc.sync.dma_start(
                        vv[:, 0:P // 128, off:off + 64],
                        pastV[b, h].rearrange("(t p) d -> p t d", p=128))

            # ---- QKV projection for one 512-token block ----
            def emit_qkv_block(b, tc4):
                t0 = b * S + tc4 * 512
                xt = xtp.tile([128, 8, 512], BF16, name="xt")
                nc.sync.dma_start(
                    xt[:], xT[:, t0:t0 + 512].rearrange("(d p) t -> p d t", p=128))
                for g in range(3):  # 0=q, 1=k, 2=v
                    ps = psB.tile([128, 1024], F32, tag="mm", name="qkvps")
                    for dc in range(8):
                        nc.tensor.matmul(
                            ps[:, 0:512],
                            lhsT=w1_sb[:, dc, g * 128:(g + 1) * 128],
                            rhs=xt[:, dc, :],
                            start=(dc == 0), stop=(dc == 7))
                    if g == 0:
                        nc.vector.tensor_scalar_add(
                            qT_sb[b][:, tc4 * 512:(tc4 + 1) * 512],
                            ps[:, 0:512], b1q_sb[:])
                        continue
                    if g == 1:
                        nc.vector.tensor_scalar_add(
                            kT_z[b][0][0:64, P + tc4 * 512:P + (tc4 + 1) * 512],
                            ps[0:64, 0:512], b1k_sb[0:64, :])
                        nc.vector.tensor_scalar_add(
                            kT_z[b][1][64:128, P + tc4 * 512:P + (tc4 + 1) * 512],
                            ps[64:128, 0:512], b1k_sb[64:128, :])
                    else:
                        vT_t = stg.tile([128, 512], BF16, tag="vT")
                        nc.vector.tensor_copy(vT_t[:], ps[:, 0:512])
                    if g == 1:
                        continue
                    for j in range(4):
                        t0l = tc4 * 512 + j * 128
                        if True:
                            tp = psB.tile([128, 128], BF16, tag="mm", name="tpv")
                            nc.tensor.transpose(
                                tp[:], vT_t[:, j * 128:(j + 1) * 128], ident[:])
                            st2 = stg.tile([128, 128], F32, tag="st2")
                            nc.vector.tensor_copy(st2[:], tp[:])
                            nc.sync.dma_start(
                                presV[b, :, t0l:t0l + 128, :].rearrange(
                                    "h t d -> t h d"),
                                st2[:].rearrange("t (h d) -> t h d", h=HL))
                            slot = P // 128 + tc4 * 4 + j
                            for h in range(HL):
                                off = 0 if h == 0 else 65
                                nc.vector.tensor_copy(
                                    vv2(b)[:, slot, off:off + 64],
                                    tp[:, h * 64:h * 64 + 64])

            def emit_presk(b, tc4):
                for j in range(4):
                    t0l = tc4 * 512 + j * 128
                    for h in range(HL):
                        r0 = h * 64
                        tp = psB.tile([128, 128], BF16, tag="mm", name="tpk")
                        nc.tensor.transpose(
                            tp[:, 0:64],
                            kT_z[b][h][r0:r0 + 64, P + t0l:P + t0l + 128],
                            ident[r0:r0 + 64, r0:r0 + 64])
                        st = stg.tile([128, 64], F32, tag="st")
                        nc.vector.tensor_copy(st[:], tp[:, 0:64])
                        nc.sync.dma_start(
                            presK[b, h, t0l:t0l + 128, :], st[:])

            def emit_scores(b, h, qbase, kc):
                kn0 = kc * 128 - P
                qlo = max(qbase, kn0)
                sc = psB.tile([128, 1024], F32, tag="mm", name=f"sc{b}{h}")
                for ss in (qbase, qbase + 512):
                    s = max(ss, qlo)
                    n = ss + 512 - s
                    if n <= 0:
                        continue
                    nc.tensor.matmul(
                        sc[:, s - qbase:s - qbase + n],
                        lhsT=kT_z[b][h][:, kc * 128:(kc + 1) * 128],
                        rhs=qT_sb[b][:, s:s + n],
                        start=True, stop=True)
                return sc

            # ---- attention for one (batch, head, 1024-query tile) unit;
            # kc loop software-pipelined so PE never waits on the exp ----
            def emit_attn_unit(b, h, qh):
                qbase = qh * 1024
                kc_end = min(NKC, (P + qbase + 1024) // 128)
                apsum = psA.tile([128, 1024], F32, tag="acc")
                sc = emit_scores(b, h, qbase, 0)
                for kc in range(kc_end):
                    kn0 = kc * 128 - P
                    qlo = max(qbase, kn0)
                    et = expp.tile([128, 1024], BF16, name="et")
                    nc.scalar.activation(
                        out=et[:, qlo - qbase:1024],
                        in_=sc[:, qlo - qbase:1024],
                        func=mybir.ActivationFunctionType.Exp,
                        scale=0.125)
                    if kc + 1 < kc_end:
                        sc = emit_scores(b, h, qbase, kc + 1)
                    if kn0 >= qbase:  # diagonal chunk: causal mask
                        nc.vector.tensor_mul(
                            et[:, kn0 - qbase:kn0 - qbase + 128],
                            et[:, kn0 - qbase:kn0 - qbase + 128],
                            umask[:])
                    voff = 0 if h == 0 else 65
                    for ss in (qbase, qbase + 512):
                        s = max(ss, qlo)
                        n = ss + 512 - s
                        if n <= 0:
                            continue
                        last_kc = min(kc_end - 1, (P + ss + 511) // 128)
                        nc.tensor.matmul(
                            apsum[:, s - qbase:s - qbase + n],
                            lhsT=vaug[b][:, kc * VSLOT + voff:
                                         kc * VSLOT + voff + 128],
                            rhs=et[:, s - qbase:s - qbase + n],
                            start=(kc == 0), stop=(kc == last_kc))
                # stash unnormalized aT (row 64 of apsum = softmax sums)
                nc.vector.tensor_copy(
                    aT_all[h * 64:(h + 1) * 64,
                           b * S + qbase:b * S + qbase + 1024],
                    apsum[0:64, :])
                su = stg.tile([1, 1024], F32, tag="sums", bufs=2)
                nc.vector.tensor_copy(su[:], apsum[64:65, :])
                nc.sync.dma_start(sums_bq[b][qh][h:h + 1, :], su[:])

            # ---- per-(batch, q-half): normalize (DVE + DMA broadcast) and
            # a 0.25MB AllToAll; per-batch output projection over both halves ----
            HTOK = 1024 // N_CORES  # 128 tokens per core per (b, qh)
            rec_dram = [[dramp.tile([2, 1024], BF16, name=f"recd{b}{qh}")
                         for qh in range(2)] for b in range(B)]
            a2a_in = [[dramp.tile([N_CORES, 128, HTOK], BF16, name=f"a2ai{b}{qh}")
                       for qh in range(2)] for b in range(B)]
            a2a_out = [[dramp.tile([N_CORES, 128, HTOK], BF16, name=f"a2ao{b}{qh}")
                        for qh in range(2)] for b in range(B)]

            def emit_norm_a2a(b, qh):
                with nc.allow_low_precision(reason="bf16 softmax recip"):
                    nc.vector.reciprocal(rec_bq[b][qh][:], sums_bq[b][qh][:])
                nc.sync.dma_start(rec_dram[b][qh].opt(), rec_bq[b][qh][:])
                rb_sb = stg.tile([128, 1024], BF16, tag="rb", bufs=2)
                for h in range(HL):
                    nc.sync.dma_start(
                        rb_sb[h * 64:(h + 1) * 64, :].rearrange(
                            "r (o t) -> r o t", o=1),
                        rec_dram[b][qh][h:h + 1, :].partition_broadcast(64))
                cols = slice(b * S + qh * 1024, b * S + qh * 1024 + 1024)
                nc.vector.tensor_mul(
                    aT_all[:, cols], aT_all[:, cols], rb_sb[:])
                nc.sync.dma_start(
                    a2a_in[b][qh].rearrange("r p t -> p r t"),
                    aT_all[:, cols].rearrange("p (r t) -> p r t", r=N_CORES))
                nc.gpsimd.collective_compute(
                    "AllToAll", mybir.AluOpType.bypass,
                    replica_groups=[list(range(N_CORES))],
                    ins=[a2a_in[b][qh].opt()], outs=[a2a_out[b][qh].opt()])

            def emit_outproj(b):
                ablk = pp.tile([128, N_CORES, TPB], BF16, name=f"ablk{b}")
                for qh in range(2):
                    nc.sync.dma_start(
                        ablk[:, :, qh * HTOK:(qh + 1) * HTOK],
                        a2a_out[b][qh].rearrange("r p t -> p r t"))
                for oc in range(8):
                    op = psB.tile([128, 1024], F32, tag="mm", name="op")
                    for fc in range(8):
                        nc.tensor.matmul(
                            op[:, 0:TPB],
                            lhsT=w2_sb[:, fc, oc * 128:(oc + 1) * 128],
                            rhs=ablk[:, fc, :],
                            start=(fc == 0), stop=(fc == 7))
                    ot = outp.tile([128, TPB], F32)
                    nc.vector.tensor_scalar_add(
                        ot[:], op[:, 0:TPB], b2e_sb[:, oc:oc + 1])
                    nc.sync.dma_start(
                        outT[oc * 128:(oc + 1) * 128, b * TPB:(b + 1) * TPB],
                        ot[:])

            # ---- emission schedule (qh-major units; each norm+a2a hides
            # under subsequent attention; presK transposes fill ACT-bound
            # attention slack) ----
            units = [(0, 0), (1, 0), (0, 1), (1, 1)]  # (h, qh) qh-major
            for tc4 in range(4):
                emit_qkv_block(0, tc4)
            emit_qkv_block(1, 0)
            emit_attn_unit(0, 0, 0)
            emit_qkv_block(1, 1)
            emit_attn_unit(0, 1, 0)
            emit_norm_a2a(0, 0)
            emit_qkv_block(1, 2)
            emit_attn_unit(0, 0, 1)
            emit_qkv_block(1, 3)
            emit_attn_unit(0, 1, 1)
            emit_norm_a2a(0, 1)
            emit_attn_unit(1, 0, 0)
            emit_presk(0, 0)
            emit_presk(0, 1)
            emit_attn_unit(1, 1, 0)
            emit_norm_a2a(1, 0)
            emit_presk(0, 2)
            emit_presk(0, 3)
            emit_outproj(0)
            emit_attn_unit(1, 0, 1)
            emit_presk(1, 0)
            emit_presk(1, 1)
            emit_attn_unit(1, 1, 1)
            emit_norm_a2a(1, 1)
            emit_presk(1, 2)
            emit_presk(1, 3)
            emit_outproj(1)

    nc.compile()
    return nc


def _get_nc():
    if "nc" not in _cache:
        _cache["nc"] = build()
    return _cache["nc"]


def kernel(x, past, w1, b1, w2, b2):
    x = np.asarray(x, dtype=np.float32)
    past = np.asarray(past, dtype=np.float32)
    w1 = np.asarray(w1, dtype=np.float32)
    b1 = np.asarray(b1, dtype=np.float32)
    w2 = np.asarray(w2, dtype=np.float32)
    b2 = np.asarray(b2, dtype=np.float32)

    xT = np.ascontiguousarray(x.reshape(TOK, D).T).astype(NPBF16)
    w2b = np.ascontiguousarray(w2).astype(NPBF16)
    b1v_full = b1[2 * D:3 * D]
    b2eff = (b2 + b1v_full.astype(np.float64) @ w2.astype(np.float64)).astype(np.float32)

    in_maps = []
    for c in range(N_CORES):
        hs = [2 * c, 2 * c + 1]
        qcols = np.concatenate([w1[:, h * DH:(h + 1) * DH] for h in hs], axis=1)
        kcols = np.concatenate([w1[:, D + h * DH:D + (h + 1) * DH] for h in hs], axis=1)
        vcols = np.concatenate([w1[:, 2 * D + h * DH:2 * D + (h + 1) * DH] for h in hs], axis=1)
        w1sc = np.concatenate([qcols, kcols, vcols], axis=1).astype(NPBF16)
        b1qc = np.concatenate([b1[h * DH:(h + 1) * DH] for h in hs])
        b1kc = np.concatenate([b1[D + h * DH:D + (h + 1) * DH] for h in hs])
        pkT = np.stack([np.concatenate(
            [past[b, 0, h].T for h in hs], axis=0) for b in range(B)]).astype(NPBF16)
        pv = np.stack([np.stack([past[b, 1, h] for h in hs])
                       for b in range(B)]).astype(NPBF16)
        in_maps.append({
            "xT": xT, "w1s": np.ascontiguousarray(w1sc),
            "b1q": np.ascontiguousarray(b1qc).astype(np.float32),
            "b1k": np.ascontiguousarray(b1kc).astype(np.float32),
            "pastKT": np.ascontiguousarray(pkT),
            "pastV": np.ascontiguousarray(pv),
            "w2": w2b, "b2eff": b2eff,
        })

    _cache["last_in_maps"] = in_maps
    nc = _get_nc()
    res = bass_utils.run_bass_kernel_spmd(
        nc, in_maps, core_ids=list(range(N_CORES)), trace=False)
    _cache["last_results"] = res

    out_flat = np.empty((TOK, D), dtype=np.float32)
    present = np.empty((B, 2, H, S, DH), dtype=np.float32)
    for c in range(N_CORES):
        r = res.results[c]
        for b in range(B):
            for qh in range(2):
                t0 = b * S + qh * 1024 + c * 128
                out_flat[t0:t0 + 128, :] = \
                    r["outT"][:, b * TPB + qh * 128:b * TPB + (qh + 1) * 128].T
        for j, h in enumerate((2 * c, 2 * c + 1)):
            present[:, 0, h] = r["presK"][:, j]
            present[:, 1, h] = r["presV"][:, j] + b1[2 * D + h * DH:2 * D + (h + 1) * DH]
    out = out_flat.reshape(B, S, D)
    return out, present
